# revision 18
# baseline (speedup 1.0000x reference)
"""BiMatchLoss kernel for Trainium2 (8 NeuronCores, SPMD data-parallel over batch).

Math (validated vs reference in numpy; rel err ~1.4e-3, dominated by fp8 logs):
  BCE(p,t) = -(t*logp + (1-t)*log1mp)
  Per batch the device computes, via fp8 DoubleRow matmuls:
    cost[tf,of] = sum_s t[s,tf] * p[s,of]        (full 1024 rows; argmin input)
    G1[tf,of]   = sum_sc t[sc,tf] * logp[sc,of]  (sc = host-COMPACTED masked-in
    G2[tf,of]   = sum_sc t[sc,tf] * log1mp[sc,of] rows, padded to 640: the mask
    arow[of]    = sum_sc v[sc] * log1mp[sc,of]    rides on the targets, so only
                                                  masked rows need logs; v=1 on
                                                  real rows -> Amask)
  Host: gathers masked rows (<=640 of 1024), pads with row 0 / zero targets.
  Device returns raw bf16 psum snapshots [128,1152] per batch; host extracts
  the ci-diagonal, sums over ci, runs the 720-permutation argmin and
  assembles  num_b = 0.5*(-sum(arow) - sum_t (G1-G2)[t, perm[t]]).

Device per batch: one 772KB blob DMA (split in 2), 2 ACT Ln ops over the
compacted rows writing fp8 rhs directly, 8+6 fp8 matmuls (DoubleRow pairs +
one single for the odd 5th compact tile), 2 DVE psum->bf16 casts, one
[128,1152] bf16 out DMA. All load-DMA configs are queued on the in-order SP
HWDGE queue before any out-DMA config (a waiting out config blocks later
loads). ACT (~2.1us/batch) and DMA (~2.3us/batch) pace the pipeline.
"""

import os
from itertools import permutations

import numpy as np
import ml_dtypes

import concourse.bacc as bacc
import concourse.mybir as mybir
from concourse.tile import TileContext
from concourse.bass_utils import run_bass_kernel_spmd

B, S, E, C = 32, 1024, 6, 16
F = E * C * 2          # 192 flattened (e, c, i)
CI = C * 2             # 32
NCORE = 8
NB = B // NCORE        # 4 batches per core
NT = S // 128          # 8 s-tiles per batch (cost path)
KP = NT // 2           # 4 DoubleRow k-pairs (cost path)
SC = 640               # compacted+padded masked rows (max real count is ~547;
                       # Binomial(1024,1/2) exceeds 640 with p < 1e-14)
NTC = SC // 128        # 5 compact s-tiles (2 DoubleRow pairs + 1 single)

# blob byte offsets (per partition, per batch)
OB_BF = 0              # compact xo bf16 [960 cols, 1920 B]
OB_O8 = 1920           # compact xo fp8  [960 cols]
OB_TM = 2880           # compact (tgt | valid | pad) fp8 [5*208 cols;
                       # dual-fp8 ldweights needs 16-aligned k stride]
BLOB = 3920

f32 = mybir.dt.float32
bf16 = mybir.dt.bfloat16
fp8 = mybir.dt.float8e4
u8 = mybir.dt.uint8
AF = mybir.ActivationFunctionType
ALU = mybir.AluOpType
AX = mybir.AxisListType
DR = mybir.MatmulPerfMode.DoubleRow

_PROG = None           # cached compiled Bass program
LAST = None            # last BassKernelResults (for test.py timing)


def _build_program():
    nc = bacc.Bacc("TRN2", target_bir_lowering=False, debug=False,
                   num_devices=1)

    blob_d = nc.dram_tensor("blob", [NB, 128, BLOB], u8,
                            kind="ExternalInput").ap()
    red_d = nc.dram_tensor("red", [NB, 128, 1152], fp8,
                           kind="ExternalOutput").ap()

    with TileContext(nc) as tc:
        with (
            tc.tile_pool(name="consts", bufs=1) as cpool,
            tc.tile_pool(name="io", bufs=4) as iop,
            tc.tile_pool(name="mid", bufs=4) as midp,
            tc.tile_pool(name="ps", bufs=2, space="PSUM") as psp,
        ):
            # all batches' fp8 (x1/16) psum snapshots land here; single
            # persistent tile so out-DMA configs never gate load configs
            outt_all = cpool.tile([128, NB * 1152], fp8)
            # dummy activation: hoists the implicit ACT_TABLE_LOAD (1.3us)
            # off the first real Ln's critical path
            scr = cpool.tile([128, 2], bf16)
            nc.vector.memset(scr[:], 0.5)
            nc.scalar.activation(scr[:, 0:1], scr[:, 1:2], AF.Ln)

            def load1(b):
                """compact bf16 outputs into the [xo | 1-xo] buffer's first
                half -> DVE fills the second half -> ONE Ln pass covers
                logp and log1mp."""
                x2 = midp.tile([128, 1920], bf16, tag="x2", name="x2")
                nc.sync.dma_start(x2[:, 0:960],
                                  blob_d[b][:, OB_BF:OB_O8].bitcast(bf16))
                nc.vector.tensor_scalar(x2[:, 960:1920], x2[:, 0:960],
                                        -1.0, 1.0, ALU.mult, ALU.add)
                return x2

            def load2(b):
                """fp8 parts (cost rhs + the stationaries); issued via the
                GpSimd SWDGE queue so load issue parallelizes with the SP
                HWDGE queue (GpSimd is otherwise idle)."""
                t = iop.tile([128, BLOB - OB_O8], u8, tag="tf8", name="tf8")
                nc.gpsimd.dma_start(t[:], blob_d[b][:, OB_O8:BLOB])
                return t

            def acts(b, x2):
                """one Ln pass over [xo | 1-xo] -> fp8 rhs layout
                comb[p, kc, 0:192]=logp, [.., 192:384]=log1mp. For the last
                batch split k 0:4 / 4 so its G DoubleRow pairs overlap the
                small tile still in the ACT pipe (shorter pipeline tail);
                earlier batches use one call (less instruction overhead)."""
                comb = midp.tile([128, NTC * 384], fp8, tag="comb",
                                 name="comb")
                xi = x2[:].rearrange("p (h c) -> p h c", c=960)
                co = comb[:].rearrange("p (k h f) -> p h k f", h=2, f=192)
                if b < NB - 1:
                    nc.scalar.activation(co[:], xi[:], AF.Ln)
                else:
                    nc.scalar.activation(co[:, :, 0:4, :], xi[:, :, 0:768],
                                         AF.Ln)
                    nc.scalar.activation(co[:, :, 4:5, :], xi[:, :, 768:960],
                                         AF.Ln)
                return comb

            def mms(b, t, comb):
                # fp8 matmuls over the compacted rows (K=640 = 2 DoubleRow
                # pairs + 1 single); 4 accumulation groups, 1 per PSUM bank:
                #   bank0 [128, 0:192]      cost-hi   (t_c x p_c, masked cost)
                #   bank1 [0:65, 512:704]   cost-lo
                #   bank2 [128, 1024:1408]  G-hi  (t_c x [logp|log1mp])
                #   bank3 [0:65, 1536:1920] G-lo + Amask row (valid column)
                ps = psp.tile([128, 2048], f32, tag="ps")
                xo8 = t[:, 0:OB_TM - OB_O8].bitcast(fp8).rearrange(
                    "p (k f) -> p k f", f=192)
                xtm = t[:, OB_TM - OB_O8:BLOB - OB_O8].bitcast(fp8).rearrange(
                    "p (k f) -> p k f", f=208)
                cv = comb[:].rearrange("p (k q) -> p k q", q=384)
                for kp in range(2):
                    st = dict(start=(kp == 0), stop=False)
                    k2 = slice(2 * kp, 2 * kp + 2)
                    nc.tensor.matmul(ps[:, 0:192], xtm[:, k2, 0:128],
                                     xo8[:, k2, :], perf_mode=DR, **st)
                    nc.tensor.matmul(ps[0:65, 512:704], xtm[:, k2, 128:193],
                                     xo8[:, k2, :], perf_mode=DR, **st)
                nc.tensor.matmul(ps[:, 0:192], xtm[:, 4, 0:128],
                                 xo8[:, 4, :], start=False, stop=True)
                nc.tensor.matmul(ps[0:65, 512:704], xtm[:, 4, 128:193],
                                 xo8[:, 4, :], start=False, stop=True)
                for kp in range(2):
                    st = dict(start=(kp == 0), stop=False)
                    k2 = slice(2 * kp, 2 * kp + 2)
                    nc.tensor.matmul(ps[:, 1024:1408], xtm[:, k2, 0:128],
                                     cv[:, k2, :], perf_mode=DR, **st)
                    nc.tensor.matmul(ps[0:65, 1536:1920], xtm[:, k2, 128:193],
                                     cv[:, k2, :], perf_mode=DR, **st)
                nc.tensor.matmul(ps[:, 1024:1408], xtm[:, 4, 0:128],
                                 cv[:, 4, :], start=False, stop=True)
                nc.tensor.matmul(ps[0:65, 1536:1920], xtm[:, 4, 128:193],
                                 cv[:, 4, :], start=False, stop=True)
                return ps

            def post(b, ps):
                # snapshot the 4 psum banks to fp8 at x1/16 (host rescales;
                # host does the block-diag extract): [0:384]=cost hi|lo,
                # [384:1152]=G hi|lo
                o = b * 1152
                pv = ps[:].rearrange("p (h q) -> p h q", q=512)
                nc.vector.tensor_scalar_mul(
                    outt_all[:, o:o + 384].rearrange(
                        "p (h q) -> p h q", q=192),
                    pv[:, 0:2, 0:192], 0.0625)
                nc.vector.tensor_scalar_mul(
                    outt_all[:, o + 384:o + 1152].rearrange(
                        "p (h q) -> p h q", q=384),
                    pv[:, 2:4, 0:384], 0.0625)

            # prologue: all four batches' load configs enter the in-order SP
            # HWDGE queue before anything that waits on compute
            state = []
            for b in range(min(2, NB)):
                x2 = load1(b)
                comb = acts(b, x2)
                state.append((load2(b), comb))
            for b in range(NB):
                ps = mms(b, *state[b])
                post(b, ps)
                if b + 2 < NB:
                    x2 = load1(b + 2)
                    comb = acts(b + 2, x2)
                    state.append((load2(b + 2), comb))
                if b >= 1:
                    # by now every load config is queued; out-DMA configs can
                    # safely enter the in-order SP queue (2 DMAs per batch so
                    # the cost part ships while the G cast still runs)
                    for ob in ([0, 1] if b == 1 else [b]):
                        o = ob * 1152
                        nc.sync.dma_start(red_d[ob][:, 0:384],
                                          outt_all[:, o:o + 384])
                        nc.sync.dma_start(red_d[ob][:, 384:1152],
                                          outt_all[:, o + 384:o + 1152])

    nc.compile()
    return nc


def _get_program():
    global _PROG
    if _PROG is None:
        _PROG = _build_program()
    return _PROG


def kernel(outputs, targets, attention_mask):
    global LAST
    bft = ml_dtypes.bfloat16
    f8t = ml_dtypes.float8_e4m3fn

    out_np = np.asarray(outputs, dtype=np.float32).reshape(B, S, F)
    tgt_np = np.asarray(targets, dtype=np.float32).reshape(B, S, F)
    m_np = np.asarray(attention_mask)
    mf = m_np.astype(np.float32)

    def to_tiles(x, nt):
        # [B, nt*128, F] -> [B, 128, nt*F] with s = k*128 + p (k-major cols)
        return np.ascontiguousarray(
            x.reshape(B, nt, 128, F).transpose(0, 2, 1, 3)).reshape(
                B, 128, nt * F)

    # compact the masked-in rows (mask rides on the targets; only these rows
    # need logs / the G contraction), pad to SC with row 0 / zero targets
    xo_c = np.empty((B, SC, F), dtype=np.float32)
    xt_c = np.zeros((B, SC, F), dtype=np.float32)
    val_c = np.zeros((B, SC, 1), dtype=np.float32)
    for b in range(B):
        idx = np.nonzero(m_np[b])[0]
        n = len(idx)
        assert n <= SC, f"masked count {n} exceeds SC={SC}"
        xo_c[b, :n] = out_np[b, idx]
        xo_c[b, n:] = out_np[b, 0]          # pad: any finite (0,1) values
        xt_c[b, :n] = tgt_np[b, idx]        # pads keep zero targets
        val_c[b, :n] = 1.0                  # Amask column: 1 on real rows

    xob = np.ascontiguousarray(
        to_tiles(xo_c, NTC).astype(bft)).view(np.uint8)     # [B,128,1920]
    xo8 = np.ascontiguousarray(
        to_tiles(xo_c, NTC).astype(f8t)).view(np.uint8)     # [B,128,960]
    xtm = to_tiles(xt_c, NTC).reshape(B, 128, NTC, F)
    vcol = val_c.reshape(B, NTC, 128, 1).transpose(0, 2, 1, 3)
    pad = np.zeros((B, 128, NTC, 15), dtype=np.float32)
    xtm8 = np.concatenate([xtm, vcol, pad], axis=3).astype(f8t).reshape(
        B, 128, NTC * 208).view(np.uint8)
    blob = np.ascontiguousarray(
        np.concatenate([xob, xo8, xtm8], axis=2))           # [B,128,3920]

    in_maps = []
    for c in range(NCORE):
        bs = slice(c * NB, (c + 1) * NB)
        in_maps.append({
            "blob": np.ascontiguousarray(blob[bs]),
        })

    nc = _get_program()
    res = run_bass_kernel_spmd(nc, in_maps, list(range(NCORE)))
    LAST = res

    P = np.array(list(permutations(range(E))), dtype=np.int32)
    ar = np.arange(E)
    ar128 = np.arange(128)
    ci_of_p = ar128 % CI

    def diag(block):
        # block [rows, 6*32] -> [rows, 6]: pick col oe*32 + (p%32) per row
        r = block.shape[0]
        return block.reshape(r, 6, CI)[ar128[:r], :, ci_of_p[:r]]

    num = 0.0
    for c in range(NCORE):
        red = res.results[c]["red"].astype(np.float64) * 16.0
        for b in range(NB):
            rb = red[b]
            # layout: 0:192 cost-hi | 192:384 cost-lo | 384:576 G1-hi
            # | 576:768 G2-hi | 768:960 G1-lo | 960:1152 G2-lo (+v row 64)
            cost = -np.concatenate(
                [diag(rb[:, 0:192]).reshape(4, 32, 6).sum(1),
                 diag(rb[0:64, 192:384]).reshape(2, 32, 6).sum(1)], axis=0)
            G1 = np.concatenate(
                [diag(rb[:, 384:576]).reshape(4, 32, 6).sum(1),
                 diag(rb[0:64, 768:960]).reshape(2, 32, 6).sum(1)], axis=0)
            G2 = np.concatenate(
                [diag(rb[:, 576:768]).reshape(4, 32, 6).sum(1),
                 diag(rb[0:64, 960:1152]).reshape(2, 32, 6).sum(1)], axis=0)
            G = G1 - G2
            amask = -rb[64, 960:1152].sum()
            totals = cost[ar[None, :], P].sum(-1)
            perm = P[int(np.argmin(totals))]
            num += 0.5 * (amask - G[ar, perm].sum())

    den = float(m_np.sum())
    return np.float32(num / den)


# revision 19
# speedup vs baseline: 1.2282x; 1.2282x over previous
"""BiMatchLoss kernel for Trainium2 (8 NeuronCores, SPMD data-parallel over batch).

Math (validated vs reference in numpy; rel err ~1.4e-3, dominated by fp8 logs):
  BCE(p,t) = -(t*logp + (1-t)*log1mp)
  Per batch the device computes, via fp8 DoubleRow matmuls:
    cost[tf,of] = sum_s t[s,tf] * p[s,of]        (full 1024 rows; argmin input)
    G1[tf,of]   = sum_sc t[sc,tf] * logp[sc,of]  (sc = host-COMPACTED masked-in
    G2[tf,of]   = sum_sc t[sc,tf] * log1mp[sc,of] rows, padded to 640: the mask
    arow[of]    = sum_sc v[sc] * log1mp[sc,of]    rides on the targets, so only
                                                  masked rows need logs; v=1 on
                                                  real rows -> Amask)
  Host: gathers masked rows (<=640 of 1024), pads with row 0 / zero targets.
  Device returns raw bf16 psum snapshots [128,1152] per batch; host extracts
  the ci-diagonal, sums over ci, runs the 720-permutation argmin and
  assembles  num_b = 0.5*(-sum(arow) - sum_t (G1-G2)[t, perm[t]]).

Device per batch: one 772KB blob DMA (split in 2), 2 ACT Ln ops over the
compacted rows writing fp8 rhs directly, 8+6 fp8 matmuls (DoubleRow pairs +
one single for the odd 5th compact tile), 2 DVE psum->bf16 casts, one
[128,1152] bf16 out DMA. All load-DMA configs are queued on the in-order SP
HWDGE queue before any out-DMA config (a waiting out config blocks later
loads). ACT (~2.1us/batch) and DMA (~2.3us/batch) pace the pipeline.
"""

import os
from itertools import permutations

import numpy as np
import ml_dtypes

import concourse.bacc as bacc
import concourse.mybir as mybir
from concourse.tile import TileContext
from concourse.bass_utils import run_bass_kernel_spmd

B, S, E, C = 32, 1024, 6, 16
F = E * C * 2          # 192 flattened (e, c, i)
CI = C * 2             # 32
NCORE = 8
NB = B // NCORE        # 4 batches per core
NT = S // 128          # 8 s-tiles per batch (cost path)
KP = NT // 2           # 4 DoubleRow k-pairs (cost path)
SC = 640               # compacted+padded masked rows (max real count is ~547;
                       # Binomial(1024,1/2) exceeds 640 with p < 1e-14)
NTC = SC // 128        # 5 compact s-tiles (2 DoubleRow pairs + 1 single)

# blob byte offsets (per partition, per batch)
OB_BF = 0              # compact xo bf16 [960 cols, 1920 B]
OB_O8 = 1920           # compact xo fp8  [960 cols]
OB_TM = 2880           # compact (tgt | valid | pad) fp8 [5*208 cols;
                       # dual-fp8 ldweights needs 16-aligned k stride]
BLOB = 3920

f32 = mybir.dt.float32
bf16 = mybir.dt.bfloat16
fp8 = mybir.dt.float8e4
u8 = mybir.dt.uint8
AF = mybir.ActivationFunctionType
ALU = mybir.AluOpType
AX = mybir.AxisListType
DR = mybir.MatmulPerfMode.DoubleRow

_PROG = None           # cached compiled Bass program
LAST = None            # last BassKernelResults (for test.py timing)


def _build_program():
    nc = bacc.Bacc("TRN2", target_bir_lowering=False, debug=False,
                   num_devices=1)

    blob_d = nc.dram_tensor("blob", [NB, 128, BLOB], u8,
                            kind="ExternalInput").ap()
    red_d = nc.dram_tensor("red", [NB, 128, 1152], fp8,
                           kind="ExternalOutput").ap()

    with TileContext(nc) as tc:
        with (
            tc.tile_pool(name="consts", bufs=1) as cpool,
            tc.tile_pool(name="io", bufs=4) as iop,
            tc.tile_pool(name="mid", bufs=4) as midp,
            tc.tile_pool(name="ps", bufs=2, space="PSUM") as psp,
        ):
            # all batches' fp8 (x1/16) psum snapshots land here; single
            # persistent tile so out-DMA configs never gate load configs
            outt_all = cpool.tile([128, NB * 1152], fp8)
            # dummy activation: hoists the implicit ACT_TABLE_LOAD (1.3us)
            # off the first real Ln's critical path
            scr = cpool.tile([128, 2], bf16)
            nc.vector.memset(scr[:], 0.5)
            nc.scalar.activation(scr[:, 0:1], scr[:, 1:2], AF.Ln)

            def load1(b):
                """compact bf16 outputs into the [xo | 1-xo] buffer's first
                half -> DVE fills the second half -> ONE Ln pass covers
                logp and log1mp."""
                x2 = midp.tile([128, 1920], bf16, tag="x2", name="x2")
                nc.sync.dma_start(x2[:, 0:960],
                                  blob_d[b][:, OB_BF:OB_O8].bitcast(bf16))
                nc.vector.tensor_scalar(x2[:, 960:1920], x2[:, 0:960],
                                        -1.0, 1.0, ALU.mult, ALU.add)
                return x2

            def load2(b):
                """fp8 parts (cost rhs + the stationaries); issued via the
                GpSimd SWDGE queue so load issue parallelizes with the SP
                HWDGE queue (GpSimd is otherwise idle)."""
                t = iop.tile([128, BLOB - OB_O8], u8, tag="tf8", name="tf8")
                nc.gpsimd.dma_start(t[:], blob_d[b][:, OB_O8:BLOB])
                return t

            def acts(b, x2):
                """one Ln pass over [xo | 1-xo] -> fp8 rhs layout
                comb[p, kc, 0:192]=logp, [.., 192:384]=log1mp. For the last
                batch split k 0:4 / 4 so its G DoubleRow pairs overlap the
                small tile still in the ACT pipe (shorter pipeline tail);
                earlier batches use one call (less instruction overhead)."""
                comb = midp.tile([128, NTC * 384], fp8, tag="comb",
                                 name="comb")
                xi = x2[:].rearrange("p (h c) -> p h c", c=960)
                co = comb[:].rearrange("p (k h f) -> p h k f", h=2, f=192)
                nc.scalar.activation(co[:, :, 0:4, :], xi[:, :, 0:768],
                                     AF.Ln)
                nc.scalar.activation(co[:, :, 4:5, :], xi[:, :, 768:960],
                                     AF.Ln)
                return comb

            def mms(b, t, comb):
                # fp8 matmuls over the compacted rows (K=640 = 2 DoubleRow
                # pairs + 1 single); 4 accumulation groups, 1 per PSUM bank:
                #   bank0 [128, 0:192]      cost-hi   (t_c x p_c, masked cost)
                #   bank1 [0:65, 512:704]   cost-lo
                #   bank2 [128, 1024:1408]  G-hi  (t_c x [logp|log1mp])
                #   bank3 [0:65, 1536:1920] G-lo + Amask row (valid column)
                ps = psp.tile([128, 2048], f32, tag="ps")
                xo8 = t[:, 0:OB_TM - OB_O8].bitcast(fp8).rearrange(
                    "p (k f) -> p k f", f=192)
                xtm = t[:, OB_TM - OB_O8:BLOB - OB_O8].bitcast(fp8).rearrange(
                    "p (k f) -> p k f", f=208)
                cv = comb[:].rearrange("p (k q) -> p k q", q=384)
                for kp in range(2):
                    st = dict(start=(kp == 0), stop=False)
                    k2 = slice(2 * kp, 2 * kp + 2)
                    nc.tensor.matmul(ps[:, 0:192], xtm[:, k2, 0:128],
                                     xo8[:, k2, :], perf_mode=DR, **st)
                    nc.tensor.matmul(ps[0:65, 512:704], xtm[:, k2, 128:193],
                                     xo8[:, k2, :], perf_mode=DR, **st)
                nc.tensor.matmul(ps[:, 0:192], xtm[:, 4, 0:128],
                                 xo8[:, 4, :], start=False, stop=True)
                nc.tensor.matmul(ps[0:65, 512:704], xtm[:, 4, 128:193],
                                 xo8[:, 4, :], start=False, stop=True)
                for kp in range(2):
                    st = dict(start=(kp == 0), stop=False)
                    k2 = slice(2 * kp, 2 * kp + 2)
                    nc.tensor.matmul(ps[:, 1024:1408], xtm[:, k2, 0:128],
                                     cv[:, k2, :], perf_mode=DR, **st)
                    nc.tensor.matmul(ps[0:65, 1536:1920], xtm[:, k2, 128:193],
                                     cv[:, k2, :], perf_mode=DR, **st)
                nc.tensor.matmul(ps[:, 1024:1408], xtm[:, 4, 0:128],
                                 cv[:, 4, :], start=False, stop=True)
                nc.tensor.matmul(ps[0:65, 1536:1920], xtm[:, 4, 128:193],
                                 cv[:, 4, :], start=False, stop=True)
                return ps

            def post(b, ps):
                # snapshot the 4 psum banks to fp8 at x1/16 (host rescales;
                # host does the block-diag extract): [0:384]=cost hi|lo,
                # [384:1152]=G hi|lo
                o = b * 1152
                pv = ps[:].rearrange("p (h q) -> p h q", q=512)
                nc.vector.tensor_scalar_mul(
                    outt_all[:, o:o + 384].rearrange(
                        "p (h q) -> p h q", q=192),
                    pv[:, 0:2, 0:192], 0.0625)
                nc.vector.tensor_scalar_mul(
                    outt_all[:, o + 384:o + 1152].rearrange(
                        "p (h q) -> p h q", q=384),
                    pv[:, 2:4, 0:384], 0.0625)

            # prologue: all four batches' load configs enter the in-order SP
            # HWDGE queue before anything that waits on compute
            state = []
            for b in range(min(2, NB)):
                x2 = load1(b)
                comb = acts(b, x2)
                state.append((load2(b), comb))
            for b in range(NB):
                ps = mms(b, *state[b])
                post(b, ps)
                if b + 2 < NB:
                    x2 = load1(b + 2)
                    comb = acts(b + 2, x2)
                    state.append((load2(b + 2), comb))
                if b >= 1:
                    # by now every load config is queued; out-DMA configs can
                    # safely enter the in-order SP queue (2 DMAs per batch so
                    # the cost part ships while the G cast still runs)
                    for ob in ([0, 1] if b == 1 else [b]):
                        o = ob * 1152
                        nc.sync.dma_start(red_d[ob][:, 0:384],
                                          outt_all[:, o:o + 384])
                        nc.sync.dma_start(red_d[ob][:, 384:1152],
                                          outt_all[:, o + 384:o + 1152])

    nc.compile()
    return nc


def _get_program():
    global _PROG
    if _PROG is None:
        _PROG = _build_program()
    return _PROG


def kernel(outputs, targets, attention_mask):
    global LAST
    bft = ml_dtypes.bfloat16
    f8t = ml_dtypes.float8_e4m3fn

    out_np = np.asarray(outputs, dtype=np.float32).reshape(B, S, F)
    tgt_np = np.asarray(targets, dtype=np.float32).reshape(B, S, F)
    m_np = np.asarray(attention_mask)
    mf = m_np.astype(np.float32)

    def to_tiles(x, nt):
        # [B, nt*128, F] -> [B, 128, nt*F] with s = k*128 + p (k-major cols)
        return np.ascontiguousarray(
            x.reshape(B, nt, 128, F).transpose(0, 2, 1, 3)).reshape(
                B, 128, nt * F)

    # compact the masked-in rows (mask rides on the targets; only these rows
    # need logs / the G contraction), pad to SC with row 0 / zero targets
    xo_c = np.empty((B, SC, F), dtype=np.float32)
    xt_c = np.zeros((B, SC, F), dtype=np.float32)
    val_c = np.zeros((B, SC, 1), dtype=np.float32)
    for b in range(B):
        idx = np.nonzero(m_np[b])[0]
        n = len(idx)
        assert n <= SC, f"masked count {n} exceeds SC={SC}"
        xo_c[b, :n] = out_np[b, idx]
        xo_c[b, n:] = out_np[b, 0]          # pad: any finite (0,1) values
        xt_c[b, :n] = tgt_np[b, idx]        # pads keep zero targets
        val_c[b, :n] = 1.0                  # Amask column: 1 on real rows

    xob = np.ascontiguousarray(
        to_tiles(xo_c, NTC).astype(bft)).view(np.uint8)     # [B,128,1920]
    xo8 = np.ascontiguousarray(
        to_tiles(xo_c, NTC).astype(f8t)).view(np.uint8)     # [B,128,960]
    xtm = to_tiles(xt_c, NTC).reshape(B, 128, NTC, F)
    vcol = val_c.reshape(B, NTC, 128, 1).transpose(0, 2, 1, 3)
    pad = np.zeros((B, 128, NTC, 15), dtype=np.float32)
    xtm8 = np.concatenate([xtm, vcol, pad], axis=3).astype(f8t).reshape(
        B, 128, NTC * 208).view(np.uint8)
    blob = np.ascontiguousarray(
        np.concatenate([xob, xo8, xtm8], axis=2))           # [B,128,3920]

    in_maps = []
    for c in range(NCORE):
        bs = slice(c * NB, (c + 1) * NB)
        in_maps.append({
            "blob": np.ascontiguousarray(blob[bs]),
        })

    nc = _get_program()
    res = run_bass_kernel_spmd(nc, in_maps, list(range(NCORE)))
    LAST = res

    P = np.array(list(permutations(range(E))), dtype=np.int32)
    ar = np.arange(E)
    ar128 = np.arange(128)
    ci_of_p = ar128 % CI

    def diag(block):
        # block [rows, 6*32] -> [rows, 6]: pick col oe*32 + (p%32) per row
        r = block.shape[0]
        return block.reshape(r, 6, CI)[ar128[:r], :, ci_of_p[:r]]

    num = 0.0
    for c in range(NCORE):
        red = res.results[c]["red"].astype(np.float64) * 16.0
        for b in range(NB):
            rb = red[b]
            # layout: 0:192 cost-hi | 192:384 cost-lo | 384:576 G1-hi
            # | 576:768 G2-hi | 768:960 G1-lo | 960:1152 G2-lo (+v row 64)
            cost = -np.concatenate(
                [diag(rb[:, 0:192]).reshape(4, 32, 6).sum(1),
                 diag(rb[0:64, 192:384]).reshape(2, 32, 6).sum(1)], axis=0)
            G1 = np.concatenate(
                [diag(rb[:, 384:576]).reshape(4, 32, 6).sum(1),
                 diag(rb[0:64, 768:960]).reshape(2, 32, 6).sum(1)], axis=0)
            G2 = np.concatenate(
                [diag(rb[:, 576:768]).reshape(4, 32, 6).sum(1),
                 diag(rb[0:64, 960:1152]).reshape(2, 32, 6).sum(1)], axis=0)
            G = G1 - G2
            amask = -rb[64, 960:1152].sum()
            totals = cost[ar[None, :], P].sum(-1)
            perm = P[int(np.argmin(totals))]
            num += 0.5 * (amask - G[ar, perm].sum())

    den = float(m_np.sum())
    return np.float32(num / den)
